# revision 27
# baseline (speedup 1.0000x reference)
"""Trainium2 Bass kernel for MAS-LoRA linear (moe_routing).

Reference computation (per batch element b):
    out[b] = x[b] @ W_base.T + b_base
             + SCALING * sum_e w[b,e] * (x[b] @ As[e].T) @ Bs[e].T

Strategy: data-parallel over batch across 8 cores (2 batch elements per
core).  Per batch element we fold the LoRA term into an effective weight
    W_eff.T[c,o] = W_base.T[c,o] + sum_er A_all[er,c] * (SCALING*w_b[er]) * B_all[er,o]
(A_all = As reshaped [E*R, C], B_all = Bs transposed to [E*R, O]), which
costs one rank-128 matmul per batch element, then a single fused GEMM
    outT[o, t] = sum_c W_eff.T[c, o] * xT[c, t] + b_base[o]
computed transposed (tokens streaming, weights stationary) in fp32r
(TF32-style) precision.  Host transposes x/out; that is part of the
shard/unshard step.
"""

import numpy as np

import concourse.bass as bass
import concourse.mybir as mybir
import concourse.tile as tile
from concourse.bass_utils import run_bass_kernel_spmd

FP32 = mybir.dt.float32
FP32R = mybir.dt.float32r

# Problem shapes (hardcoded per contract)
B, T, C, O, E, R = 16, 1500, 1024, 1024, 8, 16
ER = E * R  # 128
SCALING = 32.0 / 16.0  # alpha / r = 2.0
NCORES = 8
BPC = B // NCORES       # batch elems per core = 2
TPC = BPC * T           # tokens per core = 3000
CS = 500                # token chunk size (3 chunks per batch element)
NCH = T // CS           # chunks per batch element
CT = C // 128           # 8 c tiles
OT = O // 128           # 8 o tiles

_counter = [0]


def _split_multi_waits(nc):
    """This walrus build supports one sync-wait command per instruction;
    Tile can emit several.  Hoist extras onto single-wait NoOps just before
    the instruction (same engine => identical semantics)."""
    for fn in nc.m.functions:
        for blk in fn.blocks:
            insts = blk.instructions
            if not any(
                i.sync_info and len(i.sync_info.on_wait) > 1 for i in insts
            ):
                continue
            out = []
            for inst in insts:
                si = inst.sync_info
                if si is not None and len(si.on_wait) > 1:
                    waits = list(si.on_wait)
                    for w in waits[:-1]:
                        _counter[0] += 1
                        out.append(
                            mybir.InstNoOp(
                                name=f"waitsplit-{_counter[0]}",
                                engine=inst.engine,
                                ins=[],
                                outs=[],
                                sync_info=mybir.SyncInfo(on_wait=[w], on_update=[]),
                            )
                        )
                    si.on_wait = [waits[-1]]
                out.append(inst)
            blk.instructions = out
    return nc


def build_nc(split=True, n_iter=1, serial=False, pso_bufs=4, xin_bufs=2,
             out_bufs=2, bias_mode="act", cs=CS, wt_split=False,
             store_halves=False, weff_bufs=2 * CT, cs_first=None, cs_last=None):
    nc = bass.Bass()
    xT_d = nc.declare_dram_parameter("xT", [C, TPC], FP32R, isOutput=False)
    WT_d = nc.declare_dram_parameter("WT", [C, O], FP32, isOutput=False)
    A_d = nc.declare_dram_parameter("A", [ER, C], FP32R, isOutput=False)
    B_d = nc.declare_dram_parameter("Bm", [ER, O], FP32, isOutput=False)
    bcol_d = nc.declare_dram_parameter("bcol", [128, OT], FP32, isOutput=False)
    wcol_d = nc.declare_dram_parameter("wcol", [128, BPC], FP32, isOutput=False)
    outT_d = nc.declare_dram_parameter("outT", [O, TPC], FP32, isOutput=True)

    xT_r = xT_d.rearrange("(ct cp) t -> cp ct t", cp=128)
    WT_r = WT_d.rearrange("(ct cp) o -> cp ct o", cp=128)
    outT_r = outT_d.rearrange("(ot op) t -> op ot t", op=128)

    with tile.TileContext(nc) as tc:
        with (
            tc.tile_pool(name="const", bufs=1) as constp,
            tc.tile_pool(name="weff", bufs=weff_bufs) as weffp,
            tc.tile_pool(name="bw", bufs=2) as bwp,
            tc.tile_pool(name="xin", bufs=xin_bufs) as xinp,
            tc.tile_pool(name="outs", bufs=out_bufs) as outp,
            tc.tile_pool(name="psw", bufs=2, space="PSUM") as pswp,
            tc.tile_pool(name="pso", bufs=pso_bufs, space="PSUM") as psop,
        ):
            wcol_sb = constp.tile([128, BPC], FP32)
            nc.sync.dma_start(wcol_sb[:], wcol_d[:])
            B_sb = constp.tile([128, O], FP32)
            nc.sync.dma_start(B_sb[:], B_d[:])
            A_sb = constp.tile([128, C], FP32R)
            nc.sync.dma_start(A_sb[:], A_d[:])
            bcol_sb = constp.tile([128, OT], FP32)
            nc.sync.dma_start(bcol_sb[:], bcol_d[:])
            xt0 = None
            if wt_split:
                # first x chunk before the bulky WT load so PE can start sooner
                cs0 = cs_first[0] if cs_first is not None else cs
                xt0 = xinp.tile([128, CT, cs0], FP32R, tag="xt", name="xt_pre")
                nc.sync.dma_start(xt0[:], xT_r[:, :, 0:cs0])
            WT_sb = constp.tile([128, CT, O], FP32)
            if wt_split:
                for ct in range(CT):
                    nc.sync.dma_start(WT_sb[:, ct, :], WT_r[:, ct, :])
            else:
                nc.sync.dma_start(WT_sb[:], WT_r[:])

            for it in range(n_iter):
              if serial and it > 0:
                  tc.strict_bb_all_engine_barrier()
              for b in range(BPC):
                # Bw[er, o] = B_all[er, o] * (SCALING * w_b[er])
                bw = bwp.tile([128, O], FP32R, tag="bw", name=f"bw{it}_{b}")
                nc.vector.tensor_scalar_mul(bw[:], B_sb[:], wcol_sb[:, b : b + 1])

                # W_eff.T tiles: [c_part, o] per ct
                weff = [
                    weffp.tile([128, O], FP32R, tag="weff", name=f"weff{it}_{b}_{ct}")
                    for ct in range(CT)
                ]
                for ct in range(CT):
                    for h in range(2):
                        psw = pswp.tile([128, 512], FP32, tag="psw")
                        nc.tensor.matmul(
                            psw[:],
                            A_sb[:, ct * 128 : (ct + 1) * 128],
                            bw[:, h * 512 : (h + 1) * 512],
                            start=True,
                            stop=True,
                        )
                        nc.vector.tensor_add(
                            weff[ct][:, h * 512 : (h + 1) * 512],
                            psw[:],
                            WT_sb[:, ct, h * 512 : (h + 1) * 512],
                        )

                if cs_first is not None and b == 0:
                    plan = list(cs_first)
                elif cs_last is not None and b == BPC - 1:
                    plan = list(cs_last)
                else:
                    plan = [cs] * (T // cs)
                assert sum(plan) == T
                plan_off = [b * T + sum(plan[:i]) for i in range(len(plan))]
                for ch, csz in enumerate(plan):
                    t0 = plan_off[ch]
                    if it == 0 and b == 0 and ch == 0 and xt0 is not None:
                        xt = xt0
                    else:
                        xt = xinp.tile([128, CT, csz], FP32R, tag="xt")
                        nc.sync.dma_start(xt[:], xT_r[:, :, t0 : t0 + csz])

                    osb = outp.tile([128, OT, csz], FP32, tag="osb")
                    for ot in range(OT):
                        pso = psop.tile([128, csz], FP32, tag="pso")
                        for ct in range(CT):
                            nc.tensor.matmul(
                                pso[:],
                                weff[ct][:, ot * 128 : (ot + 1) * 128],
                                xt[:, ct, :],
                                start=(ct == 0),
                                stop=(ct == CT - 1),
                            )
                        use_act = bias_mode == "act" or (
                            bias_mode == "mix" and ot % 2 == 0
                        )
                        if use_act:
                            nc.scalar.activation(
                                osb[:, ot, :],
                                pso[:],
                                mybir.ActivationFunctionType.Identity,
                                bias=bcol_sb[:, ot : ot + 1],
                            )
                        else:
                            nc.vector.tensor_scalar_add(
                                osb[:, ot, :], pso[:], bcol_sb[:, ot : ot + 1]
                            )
                    if store_halves:
                        nc.sync.dma_start(
                            outT_r[:, 0 : OT // 2, t0 : t0 + csz], osb[:, 0 : OT // 2, :]
                        )
                        nc.sync.dma_start(
                            outT_r[:, OT // 2 : OT, t0 : t0 + csz], osb[:, OT // 2 : OT, :]
                        )
                    else:
                        nc.sync.dma_start(outT_r[:, :, t0 : t0 + csz], osb[:])

    if split:
        _split_multi_waits(nc)
    return nc


_cache = {}


BEST = dict(
    wt_split=True,
    store_halves=True,
    xin_bufs=3,
    cs_first=(476, 512, 512),
    cs_last=(512, 512, 476),
)


def _get_nc():
    if "nc" not in _cache:
        _cache["nc"] = build_nc(**BEST)
    return _cache["nc"]


def kernel(x, w, W_base, b_base, As, Bs, trace=False):
    x = np.asarray(x, dtype=np.float32)
    w = np.asarray(w, dtype=np.float32)
    W_base = np.asarray(W_base, dtype=np.float32)
    b_base = np.asarray(b_base, dtype=np.float32)
    As = np.asarray(As, dtype=np.float32)
    Bs = np.asarray(Bs, dtype=np.float32)

    WT = np.ascontiguousarray(W_base.T)                      # [c, o]
    A_r = np.ascontiguousarray(As.reshape(ER, C))            # [er, c]
    B_r = np.ascontiguousarray(Bs.transpose(0, 2, 1).reshape(ER, O))  # [er, o]
    bcol = np.ascontiguousarray(b_base.reshape(OT, 128).T)   # [op, ot]

    in_maps = []
    for i in range(NCORES):
        xs = x[i * BPC : (i + 1) * BPC].reshape(TPC, C)
        xT_i = np.ascontiguousarray(xs.T)                    # [c, t]
        wcol_i = np.ascontiguousarray(
            (SCALING * np.repeat(w[i * BPC : (i + 1) * BPC], R, axis=1)).T
        )                                                    # [er, b]
        in_maps.append(
            {"xT": xT_i, "WT": WT, "A": A_r, "Bm": B_r, "bcol": bcol, "wcol": wcol_i}
        )

    nc = _get_nc()
    res = run_bass_kernel_spmd(
        nc, in_maps, list(range(NCORES)), trace=trace
    )

    out = np.empty((B, T, O), dtype=np.float32)
    for i in range(NCORES):
        outT_i = res.results[i]["outT"]                      # [o, t]
        out[i * BPC : (i + 1) * BPC] = outT_i.T.reshape(BPC, T, O)

    if trace:
        kernel.last_result = res
    return out
